# revision 1
# baseline (speedup 1.0000x reference)
"""Trainium2 Bass kernel v6 for the GNN message-passing net.

Math (arange assoc ⇒ two fused MLP streams):
    out[0:n_con]      = head(con_mlp(con_node_features))
    out[n_con:n_var]  = head(var_mlp(var_node_features[n_con:n_var]))

Design law learned from v1-v5 traces: period = stages x (mm + drain +
handoff) / psum_bufs, with drains (PSUM->SBUF bias+ReLU on Scalar/Vector)
the irreducible cost (~1.2us per 1024 cols).  v6 maximizes psum_bufs:

  - per-layer PSUM tiles of [128,1024] fp32 (2 banks) -> 3 pool bufs fit
    alongside p5 (6+2 = 8 banks) -> 3 groups in flight at 2-stage offsets
  - round-robin software pipeline: each turn advances the 3 in-flight
    groups one layer each (oldest first), so engine queues see deps that
    resolved a full turn earlier and PE work spreads evenly
  - L5 logits quadrant-packed: 2 groups x 2 col-tiles fill all 128
    partitions of one [128,512] p5 bank -> one Sigmoid per 2 groups
  - engine balance: Scalar = r1[0:768] + r3 + sigmoid, Vector =
    r1[768:] + r2 + r4  (~2.6ns/row each)
"""

import math
import os

import numpy as np

DIM = 128
TILE_N = 512
GROUP = int(os.environ.get("K_GROUP", "2"))
GT = GROUP * TILE_N
N_CORES = 8

INFLIGHT = int(os.environ.get("K_INFLIGHT", "3"))
MMBUFS = int(os.environ.get("K_MMBUFS", "3"))
SPLIT1 = int(os.environ.get("K_SPLIT1", "0"))  # r1 cols on Scalar (0=all)
SIGBATCH = int(os.environ.get("K_SIGBATCH", "2"))  # groups per sigmoid
R3ALT = int(os.environ.get("K_R3ALT", "0"))  # r3 on Vector every Nth group
DUMMY = int(os.environ.get("K_DUMMY", "0"))  # keep-warm matmuls per turn
P5BUFS = int(os.environ.get("K_P5BUFS", "2"))
WREUSE = os.environ.get("K_WREUSE", "0") == "1"  # skip LDWEIGHTS on 2nd mm
PTILES = int(os.environ.get("K_PTILES", "4"))  # psum tiles per group (1/2/4)
PREALLOC = os.environ.get("K_PREALLOC", "0") == "1"  # oldest-first tile allocs
VSPLIT = int(os.environ.get("K_VSPLIT", "0"))  # split V drains at this col
ADMIT = int(os.environ.get("K_ADMIT", "1"))  # admit next group at this stage
WARMUP = int(os.environ.get("K_WARMUP", "14"))
FBUFS = int(os.environ.get("K_FBUFS", "6"))
ABUFS = int(os.environ.get("K_ABUFS", "16"))
SBUFS = int(os.environ.get("K_SBUFS", "4"))

_NC_CACHE = {}


def _build_nc(ncon, nvar):
    """ncon/nvar: padded rows per core (multiples of GT)."""
    import concourse.mybir as mybir
    import concourse.tile as tile
    from concourse import bacc

    dt = mybir.dt
    f32 = dt.float32
    f16 = dt.float16
    AF = mybir.ActivationFunctionType
    ALU = mybir.AluOpType

    nc = bacc.Bacc()

    cfeat = nc.dram_tensor("cfeat", [2, ncon], f16, kind="ExternalInput")
    vfeat = nc.dram_tensor("vfeat", [2, nvar], f16, kind="ExternalInput")
    blob16 = nc.dram_tensor("blob16", [DIM, 800], f16, kind="ExternalInput")
    blob32 = nc.dram_tensor("blob32", [DIM, 7], f32, kind="ExternalInput")
    out_con = nc.dram_tensor("out_con", [ncon], f32, kind="ExternalOutput")
    out_var = nc.dram_tensor("out_var", [nvar], f32, kind="ExternalOutput")

    with tile.TileContext(nc) as tc:
        with (
            tc.tile_pool(name="const", bufs=1) as cpool,
            tc.tile_pool(name="feat", bufs=FBUFS) as fpool,
            tc.tile_pool(name="acts", bufs=ABUFS) as apool,
            tc.tile_pool(name="sig", bufs=SBUFS) as spool,
            tc.tile_pool(name="mm", bufs=MMBUFS, space="PSUM") as mmpool,
            tc.tile_pool(name="p5", bufs=P5BUFS, space="PSUM") as p5pool,
        ):
            import contextlib
            _stack = contextlib.ExitStack()
            dum_t = None
            if DUMMY:
                dumpool = _stack.enter_context(
                    tc.tile_pool(name="dum", bufs=1, space="PSUM")
                )
                dum_t = dumpool.tile([DIM, TILE_N], f32, tag="dum")
            b16 = cpool.tile([DIM, 800], f16, tag="b16")
            nc.sync.dma_start(b16[:, :], blob16[:, :])
            b32 = cpool.tile([DIM, 7], f32, tag="b32")
            nc.sync.dma_start(b32[:, :], blob32[:, :])

            wmc_t = b16[:, 256:384]
            wmv_t = b16[:, 384:512]
            ww2_t = b16[:, 512:640]
            ww3_t = b16[:, 640:768]
            ww4_t = b16[:, 768:800]
            bb1c_t = b32[:, 0:1]
            bb1v_t = b32[:, 1:2]
            bbmc_t = b32[:, 2:3]
            bbmv_t = b32[:, 3:4]
            bb2_t = b32[:, 4:5]
            bb3_t = b32[:, 5:6]
            bb4_t = b32[:, 6:7]

            def s_relu(t, p, bias, c0, c1):
                nc.scalar.activation(
                    t[:, c0:c1], p[:, c0:c1], AF.Relu, bias=bias[:, :]
                )

            def v_relu(t, p, bias, c0, c1):
                if 0 < VSPLIT < c1 - c0:
                    # two same-engine pieces: the first releases the next
                    # layer's first matmul while the second still streams
                    nc.vector.tensor_scalar(
                        t[:, c0 : c0 + VSPLIT], p[:, c0 : c0 + VSPLIT],
                        bias[:, :], 0.0, ALU.add, ALU.max,
                    )
                    nc.vector.tensor_scalar(
                        t[:, c0 + VSPLIT : c1], p[:, c0 + VSPLIT : c1],
                        bias[:, :], 0.0, ALU.add, ALU.max,
                    )
                else:
                    nc.vector.tensor_scalar(
                        t[:, c0:c1], p[:, c0:c1], bias[:, :], 0.0, ALU.add, ALU.max
                    )

            def emit_feat(st):
                m = st["m"]
                ft = fpool.tile([DIM, TILE_N], f16, tag="feat")
                for j in range(GROUP):
                    off = m["g0"] + j * TILE_N
                    nc.sync.dma_start(
                        ft[32 * j : 32 * j + 2, :],
                        m["feat"][:, off : off + TILE_N],
                    )
                st["ft"] = ft

            sig_state = {"p5": None, "metas": []}

            def flush_sig():
                """One sigmoid over the packed p5 quadrants + per-group DMA."""
                metas = sig_state["metas"]
                if not metas:
                    return
                p5 = sig_state["p5"]
                pc = 64 * len(metas)
                sg = spool.tile([DIM, TILE_N], f32, tag="sig")
                nc.scalar.activation(
                    sg[:pc, :], p5[:pc, :], AF.Sigmoid, bias=bb4_t[:pc, :]
                )
                for h, m in enumerate(metas):
                    nc.gpsimd.dma_start(
                        m["out"][m["g0"] : m["g0"] + GT].rearrange(
                            "(a b) -> a b", b=TILE_N
                        ),
                        sg[64 * h : 64 * h + 32 * (GROUP - 1) + 1 : 32, :],
                    )
                sig_state["p5"] = None
                sig_state["metas"] = []

            def emit_stage(st):
                m, li = st["m"], st["li"]
                if li == 0:
                    if not PREALLOC:
                        p1 = mmpool.tile([DIM, GT], f32, tag="mm")
                        st["p"] = p1
                    p1 = st["p"]
                    for j in range(GROUP):
                        nc.tensor.matmul(
                            p1[:, j * TILE_N : (j + 1) * TILE_N],
                            b16[32 * j : 32 * j + 2, m["wa1c0"] : m["wa1c0"] + DIM],
                            st["ft"][32 * j : 32 * j + 2, :],
                            start=True,
                            stop=True,
                            tile_position=(32 * j, 0),
                        )
                    t1 = apool.tile([DIM, GT], f16, tag="acts")
                    # r1 split across both engines for balance
                    if 0 < SPLIT1 < GT:
                        s_relu(t1, p1, m["b1"], 0, SPLIT1)
                        v_relu(t1, p1, m["b1"], SPLIT1, GT)
                    else:
                        s_relu(t1, p1, m["b1"], 0, GT)
                    st["t"] = t1
                elif li < 4:
                    w = (m["wm"], ww2_t, ww3_t)[li - 1]
                    bias = (m["bm"], bb2_t, bb3_t)[li - 1]
                    if not PREALLOC and (PTILES == 4 or (PTILES == 2 and li == 2)):
                        p = mmpool.tile([DIM, GT], f32, tag="mm")
                        st["p"] = p
                    else:
                        p = st["p"]
                    for j in range(GROUP):
                        inst = nc.tensor.matmul(
                            p[:, j * TILE_N : (j + 1) * TILE_N],
                            w[:, :],
                            st["t"][:, j * TILE_N : (j + 1) * TILE_N],
                            start=True,
                            stop=True,
                        )
                        if WREUSE and j > 0:
                            inst.ldweights = False
                    t = apool.tile([DIM, GT], f16, tag="acts")
                    to_s = li == 2
                    if to_s and R3ALT and st["idx"] % R3ALT == R3ALT - 1:
                        to_s = False  # shed r3 to Vector every Nth group
                    if to_s:
                        s_relu(t, p, bias, 0, GT)
                    else:
                        v_relu(t, p, bias, 0, GT)
                    st["t"] = t
                else:
                    # L5: quadrant-pack SIGBATCH groups into one p5 bank
                    if sig_state["p5"] is None:
                        p5t = p5pool.tile([DIM, TILE_N], f32, tag="p5")
                        sig_state["p5"] = p5t
                    h = len(sig_state["metas"])
                    p5 = sig_state["p5"]
                    for j in range(GROUP):
                        q = 64 * h + 32 * j
                        nc.tensor.matmul(
                            p5[q : q + 32, :],
                            ww4_t[:, :],
                            st["t"][:, j * TILE_N : (j + 1) * TILE_N],
                            start=True,
                            stop=True,
                            tile_position=(0, q),
                        )
                    sig_state["metas"].append(m)
                    if len(sig_state["metas"]) >= SIGBATCH:
                        flush_sig()
                st["li"] = li + 1

            def stream_groups(feat, n_rows, wa1c0, b1_t, wm_t, bm_t, out):
                return [
                    {
                        "feat": feat, "out": out, "wa1c0": wa1c0,
                        "b1": b1_t, "wm": wm_t, "bm": bm_t, "g0": gi * GT,
                    }
                    for gi in range(n_rows // GT)
                ]

            if WARMUP:
                wp = mmpool.tile([DIM, GT], f32, tag="mm")
                for _ in range(WARMUP):
                    nc.tensor.matmul(
                        wp[:, :TILE_N], b16[:, 0:128], b16[:, 288:800],
                        start=True, stop=True,
                    )

            con_groups = stream_groups(cfeat, ncon, 0, bb1c_t, wmc_t, bbmc_t, out_con)
            var_groups = stream_groups(vfeat, nvar, 128, bb1v_t, wmv_t, bbmv_t, out_var)
            order = []
            i = j = 0
            while i < len(con_groups) or j < len(var_groups):
                if i < len(con_groups):
                    order.append(con_groups[i]); i += 1
                if j < len(var_groups):
                    order.append(var_groups[j]); j += 1

            stream = iter(order)
            _idx = [0]

            def new_st():
                m = next(stream, None)
                if m is None:
                    return None
                st = {"m": m, "li": 0, "idx": _idx[0]}
                _idx[0] += 1
                emit_feat(st)
                return st

            # round-robin pipeline, INFLIGHT groups at 2-stage offsets
            active = []
            pending = True
            while active or pending:
                if pending and len(active) < INFLIGHT and (
                    not active or active[-1]["li"] >= ADMIT
                ):
                    st = new_st()
                    if st is None:
                        pending = False
                    else:
                        active.append(st)
                done = []
                if PREALLOC:
                    # allocate this turn's PSUM tiles oldest-first so every
                    # pool-WAR lands on a stale drain (never a younger
                    # group's), independent of the youngest-first emission
                    for st in active:
                        if st["li"] < 4:
                            p = mmpool.tile([DIM, GT], f32, tag="mm")
                            st["p"] = p
                # youngest first: the oldest group's stage is the most
                # dependency-deep; putting it last in each queue gives its
                # deps the whole turn to resolve (no head-of-line block)
                for st in reversed(active):
                    emit_stage(st)
                    if st["li"] == 5:
                        done.append(st)
                for st in done:
                    active.remove(st)
                # dependency-free keep-warm matmuls: fill this turn's PE
                # tail gap so the HAM clock gate never sees low duty
                for _ in range(DUMMY):
                    nc.tensor.matmul(
                        dum_t[:, 0:128], b16[:, 0:128], b16[:, 288:416],
                        start=True, stop=True,
                    )
            flush_sig()
            _stack.close()

    nc.compile()
    return nc


def _make_in_maps(inputs, ncon_per, nvar_per):
    f32 = np.float32
    f16 = np.float16
    cf = np.asarray(inputs["con_node_features"], f32)
    vf = np.asarray(inputs["var_node_features"], f32)
    n_con = cf.shape[0]
    n_var = vf.shape[0]

    W1 = np.asarray(inputs["W1"], f32)
    b1 = np.asarray(inputs["b1"], f32)
    mc = np.asarray(inputs["cW2"], f32) @ W1
    bmc = np.asarray(inputs["cb2"], f32) @ W1 + b1
    mv = np.asarray(inputs["vW2"], f32) @ W1
    bmv = np.asarray(inputs["vb2"], f32) @ W1 + b1

    ncp = math.ceil(n_con / N_CORES)
    nvp = math.ceil((n_var - n_con) / N_CORES)
    conT = np.zeros((N_CORES, 2, ncon_per), f16)
    cfT = cf.T
    varT = np.zeros((N_CORES, 2, nvar_per), f16)
    vfT = vf[n_con:].T
    for i in range(N_CORES):
        c = cfT[:, i * ncp : (i + 1) * ncp]
        conT[i, :, : c.shape[1]] = c
        v = vfT[:, i * nvp : (i + 1) * nvp]
        varT[i, :, : v.shape[1]] = v

    blob16 = np.zeros((DIM, 800), f16)
    for j in range(4):
        blob16[32 * j : 32 * j + 2, 0:128] = np.asarray(inputs["cW1"], f32).astype(f16)
        blob16[32 * j : 32 * j + 2, 128:256] = np.asarray(inputs["vW1"], f32).astype(f16)
    blob16[:, 256:384] = mc.astype(f16)
    blob16[:, 384:512] = mv.astype(f16)
    blob16[:, 512:640] = np.asarray(inputs["W2"], f32).astype(f16)
    blob16[:, 640:768] = np.asarray(inputs["W3"], f32).astype(f16)
    blob16[:, 768:800] = np.repeat(
        np.asarray(inputs["W4"], f32).reshape(DIM, 1), 32, axis=1
    ).astype(f16)
    blob32 = np.zeros((DIM, 7), f32)
    blob32[:, 0] = np.asarray(inputs["cb1"], f32)
    blob32[:, 1] = np.asarray(inputs["vb1"], f32)
    blob32[:, 2] = bmc
    blob32[:, 3] = bmv
    blob32[:, 4] = np.asarray(inputs["b2"], f32)
    blob32[:, 5] = np.asarray(inputs["b3"], f32)
    blob32[:, 6] = np.asarray(inputs["b4"], f32).reshape(-1)[0]

    shared = {"blob16": blob16, "blob32": blob32}
    in_maps = []
    for i in range(N_CORES):
        m = dict(shared)
        m["cfeat"] = np.ascontiguousarray(conT[i])
        m["vfeat"] = np.ascontiguousarray(varT[i])
        in_maps.append(m)
    return in_maps


def _reference_numpy(inputs):
    f32 = np.float32

    def mlp2(x, W1, b1, W2, b2):
        return np.maximum(x @ W1 + b1, 0.0) @ W2 + b2

    vf = np.asarray(inputs["var_node_features"], f32)
    cf = np.asarray(inputs["con_node_features"], f32)
    av = np.asarray(inputs["assoc_var"])
    ac = np.asarray(inputs["assoc_con"])
    n = mlp2(vf, inputs["vW1"], inputs["vb1"], inputs["vW2"], inputs["vb2"])
    e = mlp2(cf, inputs["cW1"], inputs["cb1"], inputs["cW2"], inputs["cb2"])
    x = np.zeros((np.asarray(inputs["node_types"]).shape[0], n.shape[-1]), f32)
    x[av] = n
    x[ac] = e
    x = x[av]
    x = np.maximum(x @ inputs["W1"] + inputs["b1"], 0.0)
    x = np.maximum(x @ inputs["W2"] + inputs["b2"], 0.0)
    x = np.maximum(x @ inputs["W3"] + inputs["b3"], 0.0)
    x = x @ inputs["W4"] + inputs["b4"]
    return (1.0 / (1.0 + np.exp(-x))).astype(f32).squeeze(-1)


def kernel(**inputs):
    from concourse.bass_utils import run_bass_kernel_spmd

    cf = np.asarray(inputs["con_node_features"])
    vf = np.asarray(inputs["var_node_features"])
    av = np.asarray(inputs["assoc_var"])
    ac = np.asarray(inputs["assoc_con"])
    n_con = cf.shape[0]
    n_var = vf.shape[0]

    fast = (
        n_con <= n_var
        and av.shape[0] == n_var
        and ac.shape[0] == n_con
        and np.array_equal(av, np.arange(n_var, dtype=av.dtype))
        and np.array_equal(ac, np.arange(n_con, dtype=ac.dtype))
    )
    if not fast:
        return _reference_numpy(inputs)

    ncon_per = math.ceil(n_con / N_CORES / GT) * GT
    nvar_per = math.ceil((n_var - n_con) / N_CORES / GT) * GT

    key = (ncon_per, nvar_per)
    if key not in _NC_CACHE:
        _NC_CACHE[key] = _build_nc(ncon_per, nvar_per)
    nc = _NC_CACHE[key]

    in_maps = _make_in_maps(inputs, ncon_per, nvar_per)
    res = run_bass_kernel_spmd(nc, in_maps, core_ids=list(range(N_CORES)))

    ncp = math.ceil(n_con / N_CORES)
    nvp = math.ceil((n_var - n_con) / N_CORES)
    out = np.empty(n_var, np.float32)
    oc = np.concatenate([r["out_con"][:ncp] for r in res.results])
    ov = np.concatenate([r["out_var"][:nvp] for r in res.results])
    out[:n_con] = oc[:n_con]
    out[n_con:] = ov[: n_var - n_con]
    return out

